# revision 28
# baseline (speedup 1.0000x reference)
"""Trainium2 Bass kernel for the 2D-LSTM (nn_Lstm2D) problem.

Reference computation (B=64, C=3, H=W=128, P=4 patch, NC=512 cells):
  - image is cut into a 32x32 grid of 4x4 patches, raster-scanned (1024 steps)
  - per step t=(i,j):  gates = [x_t, h_prevrow_j] @ W_ih.T + h_{t-1} @ W_hh.T + b
                       i,f,g,o = split(gates); c = sig(f)*c + sig(i)*tanh(g)
                       h = sig(o)*tanh(c)
  - output: h at every grid cell -> (B, 512, 32, 32)

Strategy (8 NeuronCores, data-parallel over batch, 8 batch elements/core):
  - the x / previous-row / bias contribution of the gates is accumulated
    directly in PSUM (two 4-bank half-row regions, ping-pong), one gate-slot
    per step interleaved with the recurrence so the PE fills its idle gaps.
    The bias rides along as a 49th "ones" feature of x.
  - per-step recurrence h @ W_hh.T accumulates onto the pre-filled PSUM
    (start=False), so the elementwise phase reads finished gates straight
    from PSUM: no gates-add, no bias-add, no SBUF staging of the pre part.
  - PSUM regions are split per gate group (f | g | i | o, one bank each) so
    the activations' dependencies are group-granular and start mid-burst.
  - h is produced directly in bf16 (it is both the next-step matmul operand
    and the kernel output; the host upcasts), fp32 only for the c state.
  - fully unrolled row loop: rows alternate between two h buffers (no
    prev-row copy) and no ~6us loop-branch seams remain.
  - the Tile list-scheduler piles the pre-fill matmuls up at the half-row
    boundaries (ALAP placement), costing ~0.4ms in boundary stalls; we
    defeat it by capturing its schedule manifest and replaying with every
    block re-ordered to pure emission order (which interleaves one pre-fill
    group into each step's tensor-engine gap by construction).
"""

import json
import os
import re
import shutil
import tempfile

import numpy as np
import ml_dtypes

B = 64
C = 3
H = W = 128
P = 4
NCELL = 512
IN = C * P * P          # 48
IN1 = IN + 1            # 49: x augmented with a constant-1 row (bias)
SY = SX = 32
NCORES = 8
BL = B // NCORES        # 8 batch elements per core
KC = NCELL // 128       # 4 contraction chunks for h
MC = (4 * NCELL) // 128  # 16 gate-dim chunks
HS = SX // 2            # 16 steps per half row
RU = 32                 # rows per hardware-loop body
# gate slot order (f, g, i, o): the binding dependency chain is
# g -> tanh(g) -> i*g -> c -> tanh(c) -> h, so the g matmuls issue early
# (pairs 16-31) and o last (only needed for the final h product).
# torch gate chunk order is i(0-3), f(4-7), g(8-11), o(12-15).
SLOT_TO_ORIG = [4, 5, 6, 7, 8, 9, 10, 11, 0, 1, 2, 3, 12, 13, 14, 15]

BF16 = ml_dtypes.bfloat16


def _patch_fishpath():
    """This concourse build's FishPath shim lacks a few pathlib conveniences
    the manifest capture/replay paths need."""
    from concourse import _compat

    FP = _compat.FishPath
    if not hasattr(FP, "open"):
        def _open(self, mode="r"):
            self._path.parent.mkdir(parents=True, exist_ok=True)
            return open(self._path, mode)
        FP.open = _open
    if not hasattr(FP, "__fspath__"):
        FP.__fspath__ = lambda self: str(self._path)
    if not hasattr(FP, "parent"):
        FP.parent = property(lambda self: _compat.FishPath(self._path.parent))
    if not hasattr(FP, "stem"):
        FP.stem = property(lambda self: self._path.stem)


def _emission_order_manifest(src_dir, dst_dir):
    """Rewrite every block's instruction order to emission order (sort by
    the I-<n> instruction counter)."""
    names = [n for n in os.listdir(src_dir) if n.endswith(".json")]
    assert len(names) == 1, names
    with open(os.path.join(src_dir, names[0])) as f:
        m = json.load(f)

    def inum(e):
        return int(re.match(r"I-(\d+)", e["name"]).group(1))

    m["order"] = {blk: sorted(v, key=inum) for blk, v in m["order"].items()}
    os.makedirs(dst_dir, exist_ok=True)
    with open(os.path.join(dst_dir, names[0]), "w") as f:
        json.dump(m, f)


def _build_module():
    import concourse.bass as bass
    import concourse.bacc as bacc
    import concourse.tile as tile
    import concourse.mybir as mybir

    f32 = mybir.dt.float32
    bf16 = mybir.dt.bfloat16
    SIG = mybir.ActivationFunctionType.Sigmoid
    TANH = mybir.ActivationFunctionType.Tanh

    nc = bacc.Bacc()

    # x: per row 32 pos x 8 batch = 256 cols; padded with one zero row (row 32)
    x_d = nc.declare_dram_parameter("xt", [IN1, (SY + 1) * SX * BL], bf16,
                                    isOutput=False)
    whh_d = nc.declare_dram_parameter("whht", [128, KC * MC * 128], bf16,
                                      isOutput=False)
    wp_d = nc.declare_dram_parameter("wpt", [128, KC * MC * 128], bf16,
                                     isOutput=False)
    wx_d = nc.declare_dram_parameter("wxt", [IN1, MC * 128], bf16,
                                     isOutput=False)
    out_d = nc.declare_dram_parameter("out", [128, KC, SY * SX, BL], bf16,
                                      isOutput=True)

    with tile.TileContext(nc) as tc:
        with (
            tc.tile_pool(name="persist", bufs=1) as persist,
            tc.tile_pool(name="gates", bufs=3) as gpool,
            tc.tile_pool(name="psum", bufs=1, space="PSUM") as pspool,
        ):
            whh_sb = persist.tile([128, KC, MC, 128], bf16)
            wp_sb = persist.tile([128, KC, MC, 128], bf16)
            wx_sb = persist.tile([IN1, MC, 128], bf16)
            c_sb = persist.tile([128, KC, BL], f32)
            hbfA = persist.tile([128, KC, SX, BL], bf16)
            hbfB = persist.tile([128, KC, SX, BL], bf16)
            xs = [persist.tile([IN1, SX * BL], bf16, name=f"xr{r}")
                  for r in range(RU + 1)]
            x0 = persist.tile([IN1, HS * BL], bf16)

            # two half-row PSUM gate regions, split per gate group (one
            # 2KB bank each: f, g, i, o) so the elementwise phase's
            # dependencies are per-group and each ACT starts mid-burst.
            psA = tuple(pspool.tile([128, 4, HS, BL], f32, name=f"psA{g}")
                        for g in range(4))
            psB = tuple(pspool.tile([128, 4, HS, BL], f32, name=f"psB{g}")
                        for g in range(4))

            def slot_view(ps, s):
                """(tile, local slot) for global gate slot s."""
                return ps[s // 4], s % 4

            # whh/wx load first so the first recurrence steps can start
            # ~20us earlier; wp is only consumed via the pre-fill of the
            # second half-row (~60us in), so it loads last.
            nc.sync.dma_start(out=whh_sb[:], in_=whh_d[:])
            nc.sync.dma_start(out=wx_sb[:], in_=wx_d[:])
            nc.sync.dma_start(out=x0[:], in_=x_d[:, 0:HS * BL])
            nc.sync.dma_start(out=wp_sb[:], in_=wp_d[:])
            nc.vector.memset(c_sb[:], 0.0)
            nc.vector.memset(hbfA[:], 0.0)
            nc.vector.memset(hbfB[:], 0.0)

            # pull the sigmoid/tanh ACT table load out of the loop
            warm = persist.tile([1, 1], f32)
            nc.vector.memset(warm[:], 0.0)
            nc.scalar.activation(out=warm[:], in_=warm[:], func=SIG)
            nc.scalar.activation(out=warm[:], in_=warm[:], func=TANH)

            # start=True clears the has_written bits for the WHOLE bank, so it
            # may only be issued on the first slot of each 4-slot bank; later
            # slots overwrite (bit cleared) then accumulate (bit set).
            def prefill(ps, s, xrow, xh, hprev, hh):
                """Accumulate slot s of half-row region ps with the x/bias and
                prev-row contributions: x half xh (0/1) of xrow, h half hh of
                hprev."""
                t, ls = slot_view(ps, s)
                nc.tensor.matmul(
                    t[:, ls, :, :], wx_sb[:, s, :],
                    xrow[:, xh * HS * BL:(xh + 1) * HS * BL],
                    start=(s % 4 == 0), stop=False)
                for k in range(KC):
                    nc.tensor.matmul(
                        t[:, ls, :, :], wp_sb[:, k, s, :],
                        hprev[:, k, hh * HS:(hh + 1) * HS, :],
                        start=False, stop=False)

            # bootstrap: pre-fill region A with row 0 first half (prev row is
            # all zeros, so only the x/bias part).
            for s in range(MC):
                t, ls = slot_view(psA, s)
                nc.tensor.matmul(t[:, ls, :, :], wx_sb[:, s, :], x0[:],
                                 start=(s % 4 == 0), stop=False)

            def step(j, cur, prev, ps, scol, pre):
                """One LSTM step: recurrence matmuls accumulate onto
                ps[:][:, :, scol, :], then the elementwise phase writes h
                (bf16) into cur[:, :, j, :].  pre() queues one pre-fill slot
                of the other half-row region on the tensor engine."""
                for s in range(MC):
                    t, ls = slot_view(ps, s)
                    for k in range(KC):
                        rhs = (prev[:, k, SX - 1, :] if j == 0
                               else cur[:, k, j - 1, :])
                        nc.tensor.matmul(
                            t[:, ls, scol, :], whh_sb[:, k, s, :], rhs,
                            start=False, stop=(k == KC - 1))
                pre()

                sf = gpool.tile([128, KC, BL], f32)
                tg = gpool.tile([128, KC, BL], f32)
                si = gpool.tile([128, KC, BL], f32)
                so = gpool.tile([128, KC, BL], f32)
                tc_t = gpool.tile([128, KC, BL], f32)
                fc = gpool.tile([128, KC, BL], f32)
                ig = gpool.tile([128, KC, BL], f32)
                nc.scalar.activation(out=sf[:], in_=ps[0][:, :, scol, :],
                                     func=SIG)
                # tanh(g) in place: the g PSUM bank is dead after this
                # read, and ScalarE writes PSUM faster than SBUF.
                nc.scalar.activation(out=ps[1][:, :, scol, :],
                                     in_=ps[1][:, :, scol, :], func=TANH)
                nc.scalar.activation(out=si[:], in_=ps[2][:, :, scol, :],
                                     func=SIG)
                nc.vector.tensor_mul(fc[:], sf[:], c_sb[:])
                nc.vector.tensor_mul(ig[:], si[:], ps[1][:, :, scol, :])
                nc.vector.tensor_add(c_sb[:], fc[:], ig[:])
                nc.scalar.activation(out=so[:], in_=ps[3][:, :, scol, :],
                                     func=SIG)
                nc.scalar.activation(out=tc_t[:], in_=c_sb[:], func=TANH)
                nc.vector.tensor_mul(cur[:, :, j, :], so[:], tc_t[:])

            with tc.For_i(0, SY // RU) as iv:
                # x rows RU*iv .. RU*iv+RU (the last is the next body's first
                # row; row 32 is zero padding)
                for r in range(RU + 1):
                    nc.gpsimd.dma_start(
                        out=xs[r][:],
                        in_=x_d[:, bass.ds((iv * RU + r) * SX * BL, SX * BL)])

                for r in range(RU):
                    cur, prev = (hbfA, hbfB) if r % 2 == 0 else (hbfB, hbfA)
                    for j in range(SX):
                        ps, scol = (psA, j) if j < HS else (psB, j - HS)
                        if j < HS:
                            # pre-fill psB slot j for this row's 2nd half
                            pre = (lambda s=j, xr=xs[r], hp=prev:
                                   prefill(psB, s, xr, 1, hp, 1))
                        else:
                            # pre-fill psA slot j-16 for the next row's first
                            # half (at iv=7, r=3 this consumes the x zero
                            # padding row and is itself never consumed)
                            pre = (lambda s=j - HS, xr=xs[r + 1], hp=cur:
                                   prefill(psA, s, xr, 0, hp, 0))
                        step(j, cur, prev, ps, scol, pre)
                    nc.gpsimd.dma_start(
                        out=out_d[:, :, bass.ds((iv * RU + r) * SX, SX), :],
                        in_=cur[:])

    nc.compile()
    return nc


def _build_with_manifest_schedule():
    """Build twice: capture the legacy Tile schedule, rewrite every block to
    emission order, then replay it.  Falls back to the plain build if any
    part of the manifest machinery is unavailable."""
    saved = {k: os.environ.get(k) for k in
             ("TILE_CAPTURE_MANIFEST_PATH", "TILE_SCHEDULER",
              "TILE_LOAD_MANIFEST_PATH")}

    def restore():
        for k, v in saved.items():
            if v is None:
                os.environ.pop(k, None)
            else:
                os.environ[k] = v

    cap_dir = tempfile.mkdtemp(prefix="lstm_manifest_cap_")
    rep_dir = tempfile.mkdtemp(prefix="lstm_manifest_rep_")
    try:
        _patch_fishpath()
        os.environ.pop("TILE_SCHEDULER", None)
        os.environ.pop("TILE_LOAD_MANIFEST_PATH", None)
        os.environ["TILE_CAPTURE_MANIFEST_PATH"] = cap_dir
        _build_module()  # capture run (module discarded)
        _emission_order_manifest(cap_dir, rep_dir)
        os.environ.pop("TILE_CAPTURE_MANIFEST_PATH", None)
        os.environ["TILE_SCHEDULER"] = "manifest"
        os.environ["TILE_LOAD_MANIFEST_PATH"] = rep_dir
        return _build_module()
    except Exception:
        restore()
        return _build_module()
    finally:
        restore()
        shutil.rmtree(cap_dir, ignore_errors=True)
        shutil.rmtree(rep_dir, ignore_errors=True)


_CACHE = {}


def _get_module():
    if "m" not in _CACHE:
        _CACHE["m"] = _build_with_manifest_schedule()
    return _CACHE["m"]


def _prep_shared(W_ih, W_hh, b_ih, b_hh):
    perm = np.array(SLOT_TO_ORIG)
    wih_t = np.ascontiguousarray(W_ih.T.astype(np.float32))     # (560, 2048)
    bias = (b_ih + b_hh).astype(np.float32).reshape(MC, 128)[perm]
    wx = wih_t[:IN]                                             # (48, 2048)
    wx = wx.reshape(IN, MC, 128)[:, perm, :]
    wx = np.concatenate([wx, bias[None, :, :]], axis=0)         # (49, 16, 128)
    wx = wx.reshape(IN1, MC * 128)
    wp = wih_t[IN:]                                             # (512, 2048)
    wp = wp.reshape(KC, 128, MC, 128)[:, :, perm, :]
    wp = wp.transpose(1, 0, 2, 3).reshape(128, KC * MC * 128)
    whh = np.ascontiguousarray(W_hh.T.astype(np.float32))       # (512, 2048)
    whh = whh.reshape(KC, 128, MC, 128)[:, :, perm, :]
    whh = whh.transpose(1, 0, 2, 3).reshape(128, KC * MC * 128)
    return (wx.astype(BF16), wp.astype(BF16), whh.astype(BF16))


def _prep_x(batch):
    # xs[i, j, b, :] = patch (C,P,P) flattened, matching the reference
    xs = batch.reshape(B, C, SY, P, SX, P).transpose(2, 4, 0, 1, 3, 5)
    xs = xs.reshape(SY, SX, B, IN)
    per_core = []
    for c in range(NCORES):
        xc = xs[:, :, c * BL:(c + 1) * BL, :]          # (SY, SX, BL, IN)
        xc = xc.transpose(3, 0, 1, 2).reshape(IN, SY, SX * BL)
        xc = np.concatenate(
            [xc, np.ones((1, SY, SX * BL), np.float32)], axis=0)
        xc = np.concatenate(
            [xc, np.zeros((IN1, 1, SX * BL), np.float32)], axis=1)
        per_core.append(
            np.ascontiguousarray(xc.reshape(IN1, (SY + 1) * SX * BL))
            .astype(BF16))
    return per_core


def _run(batch, W_ih, W_hh, b_ih, b_hh, trace=False):
    from concourse.bass_utils import run_bass_kernel_spmd

    batch = np.asarray(batch, dtype=np.float32)
    wx, wp, whh = _prep_shared(
        np.asarray(W_ih), np.asarray(W_hh), np.asarray(b_ih), np.asarray(b_hh))
    xs = _prep_x(batch)

    nc = _get_module()
    in_maps = [
        {"xt": xs[c], "whht": whh, "wpt": wp, "wxt": wx}
        for c in range(NCORES)
    ]
    res = run_bass_kernel_spmd(nc, in_maps, list(range(NCORES)), trace=trace)

    outs = []
    for c in range(NCORES):
        arr = np.asarray(res.results[c]["out"]).astype(np.float32)
        # arr axes (128, KC, T, BL): reference's to_image is a raw reshape of
        # (B, T, NC) into (B, NC, SY, SX): flatten (BL, T, KC*128)->(BL, T*NC).
        arr = arr.transpose(3, 2, 1, 0).reshape(BL, NCELL, SY, SX)
        outs.append(arr)
    return np.concatenate(outs, axis=0), res


def kernel(batch, W_ih, W_hh, b_ih, b_hh):
    out, _ = _run(batch, W_ih, W_hh, b_ih, b_hh)
    return out
